# revision 8
# baseline (speedup 1.0000x reference)
"""Distributed multi-head attention kernel for 8 TRN2 NeuronCores.

Sharding: 8-way head parallel (2 heads per core), batches looped on-core.

v2 restructure vs v1 (279us):
- attn@V is V-stationary: out^T[d(+denom), i] = v_aug[j, 65].T @ ptile[j, i]
  accumulated over 16 j-chunks with N=512 streaming -- replaces 1024 tiny
  N=65 matmuls (240us of PE instruction time, LDWEIGHTS-bound) with 256
  N=512 matmuls (55us).
- QK^T dots are ROW-TILED: qT/kT are packed [h*64+d, i] (head 0 in
  partitions 0-63, head 1 in 64-127, no zero-padding), and the two heads'
  dots run CONCURRENTLY on the PE's 64-row sub-arrays via tile_position
  auto-derivation -> ST time halves.
- Softmax denominators still come from the ones-column of v_aug (row 64 of
  psO); normalization = DVE reciprocal of that row + a rank-1 PE matmul
  (ones[1,64].T @ recip[1,512]) to broadcast it across partitions + one
  DVE multiply writing the transposed, normalized output.
- The [d, i] output orientation makes the A2A receive a pure DMA (no PE
  transposes): 8 fine-grained AllToAlls (one per (batch, 512-row block),
  128KB/core each) fire as soon as each block is normalized, so all but
  the last flight hide under attention; receives land directly in attnT.
- Output projection runs in 2 chunks (batch 0's rows mid-stream, batch 1's
  at the end) so only the last A2A + quarter projection sit in the tail.
- exp ACTIVATEs ([128, 2head, 512] from 2 PSUM banks) start ~18us in: x
  loads nb-major so k/q projections chase the DMA stream, and the ACT
  exp table is preloaded with a dummy activation at t~0.

Engine budget per core: ACT (exp) 128x1.35us = 173us (the wall), PE ~155us,
DVE ~45us -- runtime ~= prologue + ACT stream + small tail.

x and the weights are cast to bf16 on the host (bf16 is the compute
precision anyway) and x arrives pre-transposed so x^T streams straight
into SBUF with contiguous DMAs.

The per-core output is the TRANSPOSED final slice [1024, 8, 64] (rows
keyed by (batch, ib, 64-row chunk)); the host transposes during assembly.
"""
import numpy as np

import concourse.bass as bass
import concourse.mybir as mybir
from concourse import bacc
import concourse.tile as tile
from concourse.bass_utils import run_bass_kernel_spmd

# problem constants (hardcoded; kernel.py must be self-contained)
B, N, DIM = 2, 2048, 1024
H, DH = 16, 64
INNER = H * DH            # 1024
SCALE = DIM ** -0.5       # 1/32  (module scales by dim**-0.5, not dim_head)
NCORES = 8
HPC = H // NCORES         # 2 heads per core
SH = HPC * DH             # 128 inner cols per core
P = 128
KO = DIM // P             # 8 contraction chunks
JC = N // P               # 16 key chunks
IB = 512                  # query block size
NIB = N // IB             # 4 query blocks per batch
NCOLL = B * NIB           # 8 collectives, one per (batch, query block)
RPC = IB // NCORES        # 64 rows per core per collective
FP32 = mybir.dt.float32
BF16 = mybir.dt.bfloat16

REPLICA_GROUPS = [[0, 1, 2, 3, 4, 5, 6, 7]]

_NC_CACHE = {}

# set by the last kernel() call when BASS_KERNEL_TRACE=1 (for test.py)
LAST_RESULTS = None


def _build():
    nc = bacc.Bacc(num_devices=NCORES)

    x_ext = nc.declare_dram_parameter("x", [B * DIM, N], BF16, isOutput=False)
    wq_ext = nc.declare_dram_parameter("wq", [DIM, SH], BF16, isOutput=False)
    wk_ext = nc.declare_dram_parameter("wk", [DIM, SH], BF16, isOutput=False)
    wv_ext = nc.declare_dram_parameter("wv", [DIM, SH], BF16, isOutput=False)
    wo_ext = nc.declare_dram_parameter("wo", [DIM, DIM], BF16, isOutput=False)
    bo_ext = nc.declare_dram_parameter("bo", [DIM], FP32, isOutput=False)
    out_ext = nc.declare_dram_parameter(
        "out", [DIM, NCOLL, RPC], FP32, isOutput=True
    )

    with tile.TileContext(nc) as tc:
        with (
            tc.tile_pool(name="consts", bufs=1) as consts,
            tc.tile_pool(name="stage", bufs=2) as stage,
            tc.tile_pool(name="xt_pool", bufs=2) as xt_pool,
            tc.tile_pool(name="pt_pool", bufs=2) as pt_pool,
            tc.tile_pool(name="nrm", bufs=2) as nrm,
            tc.tile_pool(name="nrm2", bufs=1) as nrm2,
            tc.tile_pool(name="dram", bufs=1, space="DRAM") as dram,
            tc.tile_pool(name="st_psum", bufs=2, space="PSUM") as st_psum,
            tc.tile_pool(name="o_psum", bufs=1, space="PSUM") as o_psum,
            tc.tile_pool(name="q_psum", bufs=2, space="PSUM") as q_psum,
        ):
            # persistent tensors
            wq_sb = consts.tile([P, KO, SH], BF16)
            wk_sb = consts.tile([P, KO, SH], BF16)
            wv_sb = consts.tile([P, KO, SH], BF16)
            wo_sb = consts.tile([P, KO, DIM], BF16)
            bias_sb = consts.tile([P, KO], FP32)
            ones64 = consts.tile([1, DH], FP32)
            # packed [h*64+d, i]: head h of this core in partitions h*64..
            qT = consts.tile([P, B, N], BF16)
            kT = consts.tile([P, B, N], BF16)
            v_aug = consts.tile([P, B, JC, HPC, DH + 1], BF16)
            # normalized transposed attention output [h*64+d, (b, i)]
            outT = consts.tile([P, B, NIB, NCORES, RPC], BF16)
            # received: [inner mod 128, src core(=inner/128), coll, row]
            attnT = consts.tile([P, KO, NCOLL, RPC], BF16)

            a2a_ins = [
                dram.tile([NCORES, P, RPC], BF16, name=f"a2a_in{k}")
                for k in range(NCOLL)
            ]
            a2a_outs = [
                dram.tile([NCORES, P, RPC], BF16, name=f"a2a_out{k}")
                for k in range(NCOLL)
            ]

            nc.vector.memset(v_aug[:, :, :, :, DH : DH + 1], 1.0)
            nc.vector.memset(ones64, 1.0)
            # preload the exp table during the prologue (dummy activation)
            warm = consts.tile([1, 2], FP32)
            nc.scalar.activation(
                warm, ones64[:, 0:2], mybir.ActivationFunctionType.Exp
            )

            def load_xT(b):
                """x[b]^T arrives pre-transposed from the host; nb-major
                loads so qk_proj can chase the stream."""
                xT = xt_pool.tile([P, KO, N], BF16, tag="xT", name="xT")
                for nb in range(NIB):
                    for ko in range(KO):
                        nc.sync.dma_start(
                            xT[:, ko, nb * IB : (nb + 1) * IB],
                            x_ext[
                                b * DIM + ko * P : b * DIM + (ko + 1) * P,
                                nb * IB : (nb + 1) * IB,
                            ],
                        )
                return xT

            def load_weights():
                for w_ext, w_sb in (
                    (wk_ext, wk_sb),
                    (wq_ext, wq_sb),
                    (wv_ext, wv_sb),
                ):
                    nc.scalar.dma_start(
                        w_sb, w_ext.rearrange("(ko kp) c -> kp ko c", kp=P)
                    )
                nc.scalar.dma_start(
                    wo_sb, wo_ext.rearrange("(ko kp) c -> kp ko c", kp=P)
                )
                nc.scalar.dma_start(
                    bias_sb, bo_ext.rearrange("(co cp) -> cp co", cp=P)
                )

            def qk_proj(b, xT):
                # k first (attention's dots consume kT earliest), then q
                for w_sb, dstT in ((wk_sb, kT), (wq_sb, qT)):
                    for nb in range(NIB):
                        ps = q_psum.tile([P, IB], FP32, tag="qk", name="qk_ps")
                        for ko in range(KO):
                            nc.tensor.matmul(
                                ps,
                                w_sb[:, ko, :],
                                xT[:, ko, nb * IB : (nb + 1) * IB],
                                start=(ko == 0),
                                stop=(ko == KO - 1),
                            )
                        nc.vector.tensor_copy(
                            dstT[:, b, nb * IB : (nb + 1) * IB], ps
                        )

            def v_proj(b, xT):
                for mt in range(JC):
                    ps_v = q_psum.tile([P, SH], FP32, tag="qk", name="v_ps")
                    for ko in range(KO):
                        nc.tensor.matmul(
                            ps_v,
                            xT[:, ko, mt * P : (mt + 1) * P],
                            wv_sb[:, ko, :],
                            start=(ko == 0),
                            stop=(ko == KO - 1),
                        )
                    nc.vector.tensor_copy(
                        v_aug[:, b, mt, :, 0:DH],
                        ps_v.rearrange("p (h d) -> p h d", d=DH),
                    )

            def attention_block(b, ib):
                """Row-tiled QK dots for both heads concurrently, exp on
                ACT ([128, 2, 512] per j-chunk), V-stationary attn@V
                accumulating out^T[d+denom, i] per head, then
                reciprocal+broadcast normalize into outT."""
                isl = slice(ib * IB, (ib + 1) * IB)
                ptile = pt_pool.tile(
                    [P, JC, HPC, IB], BF16, tag="pt", name="ptile"
                )
                psO = o_psum.tile([P, HPC, IB], FP32, tag="po", name="o_ps")
                for jc in range(JC):
                    st = st_psum.tile([P, HPC, IB], FP32, tag="st", name="st_ps")
                    for h in range(HPC):
                        hsl = slice(h * DH, (h + 1) * DH)
                        nc.tensor.matmul(
                            st[:, h, :],
                            kT[hsl, b, jc * P : (jc + 1) * P],
                            qT[hsl, b, isl],
                            start=True,
                            stop=True,
                        )
                    nc.scalar.activation(
                        ptile[:, jc, :, :],
                        st,
                        mybir.ActivationFunctionType.Exp,
                        scale=SCALE,
                    )
                    for h in range(HPC):
                        nc.tensor.matmul(
                            psO[0 : DH + 1, h, :],
                            v_aug[:, b, jc, h, :],
                            ptile[:, jc, h, :],
                            start=(jc == 0),
                            stop=(jc == JC - 1),
                        )
                # normalize: recip of the denominator row, gpsimd broadcast
                # across partitions (in SBUF), one DVE multiply per head
                recip = nrm.tile([1, HPC, IB], FP32, tag="recip", name="recip")
                nc.vector.reciprocal(recip, psO[DH : DH + 1, :, :])
                recipB = nrm2.tile([DH, HPC, IB], FP32, tag="rb", name="recipB")
                nc.gpsimd.partition_broadcast(recipB, recip)
                for h in range(HPC):
                    nc.vector.tensor_tensor(
                        outT[h * DH : (h + 1) * DH, b, ib, :, :],
                        psO[0:DH, h, :].rearrange(
                            "d (c i) -> d c i", c=NCORES
                        ),
                        recipB[:, h, :].rearrange("d (c i) -> d c i", c=NCORES),
                        mybir.AluOpType.mult,
                    )
                # stage + fire this block's AllToAll
                k = b * NIB + ib
                nc.gpsimd.dma_start(
                    a2a_ins[k].rearrange("c p i -> p c i"),
                    outT[:, b, ib, :, :],
                )
                nc.gpsimd.collective_compute(
                    "AllToAll",
                    mybir.AluOpType.bypass,
                    replica_groups=REPLICA_GROUPS,
                    ins=[a2a_ins[k].opt()],
                    outs=[a2a_outs[k].opt()],
                )

            def receive(k):
                nc.sync.dma_start(
                    attnT[:, :, k, :],
                    a2a_outs[k].rearrange("s p i -> p s i"),
                )

            def final_chunk(ch):
                """Output projection for collectives [4ch, 4ch+4): one
                batch's 2048 rows as [4 colls, 64 rows/coll] = N=256."""
                csl = slice(4 * ch, 4 * (ch + 1))
                for cc in range(KO):
                    ps_f = q_psum.tile([P, NIB, RPC], FP32, tag="qk", name="f_ps")
                    for ko in range(KO):
                        nc.tensor.matmul(
                            ps_f,
                            wo_sb[:, ko, cc * P : (cc + 1) * P],
                            attnT[:, ko, csl, :],
                            start=(ko == 0),
                            stop=(ko == KO - 1),
                        )
                    of = stage.tile([P, NIB, RPC], FP32, tag="of", name="of")
                    nc.vector.tensor_scalar_add(of, ps_f, bias_sb[:, cc : cc + 1])
                    nc.sync.dma_start(out_ext[cc * P : (cc + 1) * P, csl, :], of)

            # ---- program order: exp starts ~18us in; all 8 A2As fire at
            # block completion; batch-0 output projection runs mid-stream ----
            load_weights()
            xT0 = load_xT(0)
            qk_proj(0, xT0)
            v_proj(0, xT0)
            xT1 = load_xT(1)
            attention_block(0, 0)
            qk_proj(1, xT1)
            v_proj(1, xT1)
            attention_block(0, 1)
            attention_block(0, 2)
            attention_block(0, 3)
            attention_block(1, 0)
            for k in range(4):
                receive(k)
            attention_block(1, 1)
            final_chunk(0)
            attention_block(1, 2)
            attention_block(1, 3)
            for k in range(4, 8):
                receive(k)
            final_chunk(1)

    nc.finalize()
    return nc


def _get_nc():
    if "nc" not in _NC_CACHE:
        _NC_CACHE["nc"] = _build()
    return _NC_CACHE["nc"]


def kernel(**inputs) -> np.ndarray:
    import os

    import ml_dtypes

    global LAST_RESULTS

    bf16 = ml_dtypes.bfloat16
    x = np.asarray(inputs["x"], dtype=np.float32)
    W_qkv = np.asarray(inputs["W_qkv"], dtype=np.float32)
    W_out = np.asarray(inputs["W_out"], dtype=np.float32)
    b_out = np.ascontiguousarray(np.asarray(inputs["b_out"], dtype=np.float32))

    x_bf = np.ascontiguousarray(
        x.transpose(0, 2, 1).reshape(B * DIM, N).astype(bf16)
    )
    wo_bf = np.ascontiguousarray(W_out.astype(bf16))
    wqkv_bf = W_qkv.astype(bf16)

    nc = _get_nc()

    in_maps = []
    for c in range(NCORES):
        in_maps.append(
            {
                "x": x_bf,
                "wq": np.ascontiguousarray(
                    wqkv_bf[:, 0 * INNER + c * SH : 0 * INNER + (c + 1) * SH]
                ),
                "wk": np.ascontiguousarray(
                    wqkv_bf[:, 1 * INNER + c * SH : 1 * INNER + (c + 1) * SH]
                ),
                "wv": np.ascontiguousarray(
                    wqkv_bf[:, 2 * INNER + c * SH : 2 * INNER + (c + 1) * SH]
                ),
                "wo": wo_bf,
                "bo": b_out,
            }
        )

    trace = os.environ.get("BASS_KERNEL_TRACE", "0") == "1"
    res = run_bass_kernel_spmd(
        nc, in_maps, core_ids=list(range(NCORES)), trace=trace
    )
    LAST_RESULTS = res

    y = np.empty((B, N, DIM), dtype=np.float32)
    for c in range(NCORES):
        o = res.results[c]["out"]  # [DIM, NCOLL, RPC]
        for k in range(NCOLL):
            b, ib = k // NIB, k % NIB
            r0 = ib * IB + c * RPC
            y[b, r0 : r0 + RPC, :] = o[:, k, :].T
    return y


# revision 13
# speedup vs baseline: 1.1294x; 1.1294x over previous
"""Distributed multi-head attention kernel for 8 TRN2 NeuronCores.

Sharding: 8-way head parallel (2 heads per core), batches looped on-core.

Design (v3):
- attn@V is V-stationary: out^T[d(+denom), i] = v_aug[j, 65].T @ ptile[j, i]
  accumulated over 16 j-chunks with N=512 streaming (vs 1024 tiny N=65
  matmuls in v1 that were LDWEIGHTS-bound at 240us of PE time).
- QK^T dots are ROW-TILED: qT/kT are packed [h*64+d, i] (head 0 in
  partitions 0-63, head 1 in 64-127) and the two heads' dots run
  CONCURRENTLY on the PE's 64-row sub-arrays (verified ~4ns apart on HW).
- The softmax exp stream on the Scalar engine (128 ACTIVATEs x ~1.35us =
  173us) is the roofline wall; every other engine is scheduled around
  keeping it fed.  Engines are FIFO, so PE work is emitted in fine-grained
  slots: each j-chunk slot emits [ST pair][one pending closure][one filler
  unit].  Pending closures carry attn@V pairs 4 slots behind their exp
  (so they never head-of-line block on ACT) and the normalize+staging of
  the previous block; filler units are 8-matmul chunks of the qkv
  projections for batch 1 and the batch-0 output projection.
- Normalize: psO row 64 holds the denominators (ones-column of v_aug).
  psO is copied to SBUF (unnorm + denom rows) to free the single psO
  buffer quickly, then reciprocal -> rank-1 PE broadcast (ones[1,64].T @
  recip) into a q_psum bank -> one DVE multiply per head (SBUF x PSUM).
- Collectives have a ~28us ncfw latency floor regardless of size, so only
  3 AllToAlls: batch 0 (fires at half-time, hidden), blocks (1,0)+(1,1)
  (hidden under the last blocks), (1,2)+(1,3) in the tail.  The gpsimd
  queue carries ONLY staging DMAs + collective triggers (a collective
  trigger blocks the gpsimd engine until completion); receives ride sync.
- The [d, i] output orientation makes receives pure DMA (no transposes);
  the x prologue streams over 3 DMA queues so the first exp fires ~18us in.

The per-core output is the TRANSPOSED final slice [1024, 8, 64] (cols
keyed by (batch, ib, 64-row chunk)); the host transposes during assembly.
"""
from collections import deque

import numpy as np

import concourse.bass as bass
import concourse.mybir as mybir
from concourse import bacc
import concourse.tile as tile
from concourse.bass_utils import run_bass_kernel_spmd

# problem constants (hardcoded; kernel.py must be self-contained)
B, N, DIM = 2, 2048, 1024
H, DH = 16, 64
INNER = H * DH            # 1024
SCALE = DIM ** -0.5       # 1/32  (module scales by dim**-0.5, not dim_head)
NCORES = 8
HPC = H // NCORES         # 2 heads per core
SH = HPC * DH             # 128 inner cols per core
P = 128
KO = DIM // P             # 8 contraction chunks
JC = N // P               # 16 key chunks
IB = 512                  # query block size
NIB = N // IB             # 4 query blocks per batch
NCOLL = B * NIB           # 8 (batch, query-block) output blocks
RPC = IB // NCORES        # 64 rows per core per block
FP32 = mybir.dt.float32
BF16 = mybir.dt.bfloat16

REPLICA_GROUPS = [[0, 1, 2, 3, 4, 5, 6, 7]]
# collective -> list of (b, ib) blocks it carries
COLLS = [[(0, 0), (0, 1), (0, 2), (0, 3)], [(1, 0), (1, 1)], [(1, 2), (1, 3)]]

_NC_CACHE = {}

# set by the last kernel() call when BASS_KERNEL_TRACE=1 (for test.py)
LAST_RESULTS = None


def _build():
    nc = bacc.Bacc(num_devices=NCORES)

    x_ext = nc.declare_dram_parameter("x", [B * DIM, N], BF16, isOutput=False)
    wq_ext = nc.declare_dram_parameter("wq", [DIM, SH], BF16, isOutput=False)
    wk_ext = nc.declare_dram_parameter("wk", [DIM, SH], BF16, isOutput=False)
    wv_ext = nc.declare_dram_parameter("wv", [DIM, SH], BF16, isOutput=False)
    wo_ext = nc.declare_dram_parameter("wo", [DIM, DIM], BF16, isOutput=False)
    bo_ext = nc.declare_dram_parameter("bo", [DIM], FP32, isOutput=False)
    out_ext = nc.declare_dram_parameter(
        "out", [DIM, NCOLL, RPC], FP32, isOutput=True
    )

    with tile.TileContext(nc) as tc:
        with (
            tc.tile_pool(name="consts", bufs=1) as consts,
            tc.tile_pool(name="stage", bufs=2) as stage,
            tc.tile_pool(name="xt_pool", bufs=2) as xt_pool,
            tc.tile_pool(name="pt_pool", bufs=2) as pt_pool,
            tc.tile_pool(name="nrm", bufs=1) as nrm,
            tc.tile_pool(name="unm", bufs=1) as unm,
            tc.tile_pool(name="dram", bufs=1, space="DRAM") as dram,
            tc.tile_pool(name="st_psum", bufs=2, space="PSUM") as st_psum,
            tc.tile_pool(name="o_psum", bufs=1, space="PSUM") as o_psum,
            tc.tile_pool(name="q_psum", bufs=2, space="PSUM") as q_psum,
        ):
            # persistent tensors
            wq_sb = consts.tile([P, KO, SH], BF16)
            wk_sb = consts.tile([P, KO, SH], BF16)
            wv_sb = consts.tile([P, KO, SH], BF16)
            wo_sb = consts.tile([P, KO, DIM], BF16)
            bias_sb = consts.tile([P, KO], FP32)
            ones64 = consts.tile([1, DH], FP32)
            # packed [h*64+d, i]: head h of this core in partitions h*64..
            qT = consts.tile([P, B, N], BF16)
            kT = consts.tile([P, B, N], BF16)
            v_aug = consts.tile([P, B, JC, HPC, DH + 1], BF16)
            # normalized transposed attention output [h*64+d, b, ib, c, i]
            outT = consts.tile([P, B, NIB, NCORES, RPC], BF16)
            # received: [inner mod 128, src core(=inner/128), block, row]
            attnT = consts.tile([P, KO, NCOLL, RPC], BF16)

            a2a_ins = [
                dram.tile([NCORES, P, len(blks), RPC], BF16, name=f"a2a_in{k}")
                for k, blks in enumerate(COLLS)
            ]
            a2a_outs = [
                dram.tile([NCORES, P, len(blks), RPC], BF16, name=f"a2a_out{k}")
                for k, blks in enumerate(COLLS)
            ]

            # preload the exp table right away (dummy activation on ones64)
            nc.vector.memset(ones64, 1.0)
            warm = consts.tile([1, 2], FP32)
            nc.scalar.activation(
                warm, ones64[:, 0:2], mybir.ActivationFunctionType.Exp
            )
            nc.vector.memset(v_aug[:, :, :, :, DH : DH + 1], 1.0)

            # ---- input DMAs spread over 3 queues (sync/scalar/gpsimd) ----
            def load_x_nb(b, nb, xT, eng):
                for ko in range(KO):
                    eng.dma_start(
                        xT[:, ko, nb * IB : (nb + 1) * IB],
                        x_ext[
                            b * DIM + ko * P : b * DIM + (ko + 1) * P,
                            nb * IB : (nb + 1) * IB,
                        ],
                    )

            def load_w(w_ext, w_sb):
                nc.scalar.dma_start(
                    w_sb, w_ext.rearrange("(ko kp) c -> kp ko c", kp=P)
                )

            xT0 = xt_pool.tile([P, KO, N], BF16, tag="xT", name="xT0")
            xT1 = xt_pool.tile([P, KO, N], BF16, tag="xT", name="xT1")
            load_w(wk_ext, wk_sb)
            load_w(wq_ext, wq_sb)
            load_x_nb(0, 0, xT0, nc.sync)
            load_x_nb(0, 1, xT0, nc.sync)
            load_x_nb(0, 2, xT0, nc.scalar)
            load_x_nb(0, 3, xT0, nc.gpsimd)
            load_w(wv_ext, wv_sb)
            load_x_nb(1, 0, xT1, nc.sync)
            load_x_nb(1, 1, xT1, nc.sync)
            load_x_nb(1, 2, xT1, nc.scalar)
            load_x_nb(1, 3, xT1, nc.gpsimd)
            load_w(wo_ext, wo_sb)
            nc.scalar.dma_start(
                bias_sb, bo_ext.rearrange("(co cp) -> cp co", cp=P)
            )

            # ---- filler units (each ~8 matmuls + a DVE drain) ----
            def qk_unit(b, xT, w_sb, dstT, nb):
                ps = q_psum.tile([P, IB], FP32, tag="qk", name="qk_ps")
                for ko in range(KO):
                    nc.tensor.matmul(
                        ps,
                        w_sb[:, ko, :],
                        xT[:, ko, nb * IB : (nb + 1) * IB],
                        start=(ko == 0),
                        stop=(ko == KO - 1),
                    )
                nc.vector.tensor_copy(dstT[:, b, nb * IB : (nb + 1) * IB], ps)

            def v_unit(b, xT, mt):
                ps_v = q_psum.tile([P, SH], FP32, tag="qk", name="v_ps")
                for ko in range(KO):
                    nc.tensor.matmul(
                        ps_v,
                        xT[:, ko, mt * P : (mt + 1) * P],
                        wv_sb[:, ko, :],
                        start=(ko == 0),
                        stop=(ko == KO - 1),
                    )
                nc.vector.tensor_copy(
                    v_aug[:, b, mt, :, 0:DH],
                    ps_v.rearrange("p (h d) -> p h d", d=DH),
                )

            def final_unit(k, cc):
                """Output projection for collective k's blocks, one 128-col
                chunk of DIM; bias add + result DMA (sync queue)."""
                b0 = 4 * k if k < 2 else 6
                nb = len(COLLS[k])
                csl = slice(b0, b0 + nb)
                ps_f4 = q_psum.tile([P, NIB, RPC], FP32, tag="qk", name="f_ps")
                ps_f = ps_f4[:, 0:nb, :]
                for ko in range(KO):
                    nc.tensor.matmul(
                        ps_f,
                        wo_sb[:, ko, cc * P : (cc + 1) * P],
                        attnT[:, ko, csl, :],
                        start=(ko == 0),
                        stop=(ko == KO - 1),
                    )
                of4 = stage.tile([P, NIB, RPC], FP32, tag="of", name="of")
                of = of4[:, 0:nb, :]
                nc.vector.tensor_scalar_add(of, ps_f, bias_sb[:, cc : cc + 1])
                nc.sync.dma_start(out_ext[cc * P : (cc + 1) * P, csl, :], of)

            # ---- attention blocks with slot-based emission ----
            pending = deque()  # closures: attn@V pairs (trailing) + normalize

            def attnv_closure(b, ib, jc, ptile, psO):
                def emit():
                    for h in range(HPC):
                        nc.tensor.matmul(
                            psO[0 : DH + 1, h, :],
                            v_aug[:, b, jc, h, :],
                            ptile[:, jc, h, :],
                            start=(jc == 0),
                            stop=(jc == JC - 1),
                        )
                return emit

            def norm_closure(b, ib, psO, coll=None):
                def emit():
                    # free psO fast: copy unnormalized rows + denominators out
                    unnorm = unm.tile([DH, HPC, IB], BF16, tag="un", name="un")
                    dnm = nrm.tile([1, HPC, IB], FP32, tag="dn", name="dn")
                    nc.vector.tensor_copy(unnorm, psO[0:DH, :, :])
                    nc.vector.tensor_copy(dnm, psO[DH : DH + 1, :, :])
                    recip = nrm.tile([1, HPC, IB], FP32, tag="rc", name="rc")
                    nc.vector.reciprocal(recip, dnm)
                    for h in range(HPC):
                        psB = q_psum.tile([P, IB], FP32, tag="qk", name="b_ps")
                        nc.tensor.matmul(
                            psB[0:DH, :],
                            ones64,
                            recip[:, h, :],
                            start=True,
                            stop=True,
                        )
                        nc.vector.tensor_tensor(
                            outT[h * DH : (h + 1) * DH, b, ib, :, :],
                            unnorm[:, h, :].rearrange(
                                "d (c i) -> d c i", c=NCORES
                            ),
                            psB[0:DH, :].rearrange("d (c i) -> d c i", c=NCORES),
                            mybir.AluOpType.mult,
                        )
                    # stage this block into its collective's buffer
                    k, slot = coll_of[(b, ib)]
                    nc.gpsimd.dma_start(
                        a2a_ins[k][:, :, slot, :].rearrange("c p i -> p c i"),
                        outT[:, b, ib, :, :],
                    )
                    if coll is not None:
                        nc.gpsimd.collective_compute(
                            "AllToAll",
                            mybir.AluOpType.bypass,
                            replica_groups=REPLICA_GROUPS,
                            ins=[a2a_ins[coll].opt()],
                            outs=[a2a_outs[coll].opt()],
                        )
                return emit

            coll_of = {}
            for k, blks in enumerate(COLLS):
                for slot, blk in enumerate(blks):
                    coll_of[blk] = (k, slot)

            def attention_block(b, ib, fillers):
                isl = slice(ib * IB, (ib + 1) * IB)
                ptile = pt_pool.tile(
                    [P, JC, HPC, IB], BF16, tag="pt", name="ptile"
                )
                psO = o_psum.tile([P, HPC, IB], FP32, tag="po", name="o_ps")
                last = (b, ib) == COLLS[-1][-1]
                for jc in range(JC):
                    st = st_psum.tile(
                        [P, HPC, IB], FP32, tag="st", name="st_ps"
                    )
                    for h in range(HPC):
                        hsl = slice(h * DH, (h + 1) * DH)
                        nc.tensor.matmul(
                            st[:, h, :],
                            kT[hsl, b, jc * P : (jc + 1) * P],
                            qT[hsl, b, isl],
                            start=True,
                            stop=True,
                        )
                    nc.scalar.activation(
                        ptile[:, jc, :, :],
                        st,
                        mybir.ActivationFunctionType.Exp,
                        scale=SCALE,
                    )
                    pending.append(attnv_closure(b, ib, jc, ptile, psO))
                    while len(pending) > 4:
                        pending.popleft()()
                    if fillers:
                        fillers.pop(0)()
                kend = next(
                    (k for k, blks in enumerate(COLLS) if blks[-1] == (b, ib)),
                    None,
                )
                pending.append(norm_closure(b, ib, psO, coll=kend))
                if last:
                    while pending:
                        pending.popleft()()

            def receive(k):
                b0 = 4 * k if k < 2 else 6
                nb = len(COLLS[k])
                nc.sync.dma_start(
                    attnT[:, :, b0 : b0 + nb, :],
                    a2a_outs[k].rearrange("s p q i -> p s q i"),
                )

            # ---- program order ----
            for nb in range(NIB):
                qk_unit(0, xT0, wk_sb, kT, nb)
            for nb in range(NIB):
                qk_unit(0, xT0, wq_sb, qT, nb)

            attention_block(0, 0, [lambda m=m: v_unit(0, xT0, m) for m in range(JC)])
            attention_block(0, 1, [lambda n=n: qk_unit(1, xT1, wk_sb, kT, n) for n in range(NIB)]
                                  + [lambda n=n: qk_unit(1, xT1, wq_sb, qT, n) for n in range(NIB)])
            attention_block(0, 2, [lambda m=m: v_unit(1, xT1, m) for m in range(8)])
            attention_block(0, 3, [lambda m=m: v_unit(1, xT1, m) for m in range(8, JC)])
            attention_block(1, 0, [])
            receive(0)
            attention_block(1, 1, [])
            attention_block(1, 2, [lambda c=c: final_unit(0, c) for c in range(KO)])
            receive(1)
            attention_block(1, 3, [])
            # tail: output projection for collectives 1 and 2
            for cc in range(KO):
                final_unit(1, cc)
            receive(2)
            for cc in range(KO):
                final_unit(2, cc)

    nc.finalize()
    return nc


def _get_nc():
    if "nc" not in _NC_CACHE:
        _NC_CACHE["nc"] = _build()
    return _NC_CACHE["nc"]


def kernel(**inputs) -> np.ndarray:
    import os

    import ml_dtypes

    global LAST_RESULTS

    bf16 = ml_dtypes.bfloat16
    x = np.asarray(inputs["x"], dtype=np.float32)
    W_qkv = np.asarray(inputs["W_qkv"], dtype=np.float32)
    W_out = np.asarray(inputs["W_out"], dtype=np.float32)
    b_out = np.ascontiguousarray(np.asarray(inputs["b_out"], dtype=np.float32))

    x_bf = np.ascontiguousarray(
        x.transpose(0, 2, 1).reshape(B * DIM, N).astype(bf16)
    )
    wo_bf = np.ascontiguousarray(W_out.astype(bf16))
    wqkv_bf = W_qkv.astype(bf16)

    nc = _get_nc()

    in_maps = []
    for c in range(NCORES):
        in_maps.append(
            {
                "x": x_bf,
                "wq": np.ascontiguousarray(
                    wqkv_bf[:, 0 * INNER + c * SH : 0 * INNER + (c + 1) * SH]
                ),
                "wk": np.ascontiguousarray(
                    wqkv_bf[:, 1 * INNER + c * SH : 1 * INNER + (c + 1) * SH]
                ),
                "wv": np.ascontiguousarray(
                    wqkv_bf[:, 2 * INNER + c * SH : 2 * INNER + (c + 1) * SH]
                ),
                "wo": wo_bf,
                "bo": b_out,
            }
        )

    trace = os.environ.get("BASS_KERNEL_TRACE", "0") == "1"
    res = run_bass_kernel_spmd(
        nc, in_maps, core_ids=list(range(NCORES)), trace=trace
    )
    LAST_RESULTS = res

    y = np.empty((B, N, DIM), dtype=np.float32)
    for c in range(NCORES):
        o = res.results[c]["out"]  # [DIM, NCOLL, RPC]
        for k in range(NCOLL):
            b, ib = k // NIB, k % NIB
            r0 = ib * IB + c * RPC
            y[b, r0 : r0 + RPC, :] = o[:, k, :].T
    return y
